# revision 33
# baseline (speedup 1.0000x reference)
"""Self-attention block (B=16, S=1024, C=512, H=8, D=64) on 8 NeuronCores.

Data-parallel over batch: core i handles batches [2i, 2i+1]. No collectives.

Per-core device pipeline (all on-chip after the initial DMAs):
  qkv proj -> q,k feature-major [d, s], v token-major [s, d] with a ones
  column appended per head (so P@V_ext also yields the softmax row-sums);
  scores computed transposed S'[j, i] = k . q so exp(S') feeds the P@V
  matmul directly as lhsT with no transposes; softmax skips max-subtraction
  (logits bounded ~+-4, mathematically identical); deferred normalization
  divides O^T rows by the row-sum (reciprocal respread over 128 lanes via a
  DRAM bounce, broadcast back, in-place multiply); output projection
  consumes the normalized heads straight out of SBUF.

The value-path bias is folded through attention into the output bias
(b_eff = b_out + b_v @ w_out.T), exact because softmax rows sum to 1.

Scheduling: the next batch's q/k/v projection chunks are emitted between
this batch's attention head-pairs so the PE's slack under the ACT-bound
exp stream absorbs them.

Dtypes: attention core + q/k/v projections run bf16 (PE streams 2
elem/cycle); the output projection runs float32r (single-pass fp32). Set
ATTN_QK_DT=f32r for an all-f32r kernel (~1.4e-4 max rel err vs ~1.1e-3).
"""

import os
import numpy as np

import concourse.bacc as bacc
import concourse.tile as tile
import concourse.mybir as mybir
from concourse.bass_utils import run_bass_kernel_spmd

B, S, C, H, D = 16, 1024, 512, 8, 64
NCORES = 8
BPC = B // NCORES  # batches per core
F32 = mybir.dt.float32
MDT = mybir.dt.float32r if os.environ.get("ATTN_MM_DT", "f32r") == "f32r" else F32
ADT = mybir.dt.bfloat16 if os.environ.get("ATTN_QK_DT", "bf16") == "bf16" else MDT

SCJ = 8  # S/128 chunks (token/key chunks)
CCH = 4  # C/128 chunks (model-dim chunks)
FCH = 8  # (2C)/128 chunks of q|k features
VW = H * (D + 1)  # 520: v row width incl. ones column per head


def _register_ntff_hook():
    # run_bass_kernel_spmd(trace=True) under axon needs antenv.axon_hooks,
    # which is absent in this image; register the equivalent hook directly.
    import sys, types

    if "antenv.axon_hooks" in sys.modules:
        return
    try:
        import trn_agent_boot.trn_boot as tb

        hook = [None]
        mod = types.ModuleType("antenv.axon_hooks")
        mod.set_axon_ntff_profile_hook = lambda h: hook.__setitem__(0, h)
        mod.get_axon_ntff_profile_hook = lambda: hook[0]
        sys.modules["antenv.axon_hooks"] = mod
        mod.set_axon_ntff_profile_hook(
            tb._ntff_profile_via_ctypes("/opt/axon/libaxon_pjrt.so")
        )
    except Exception:
        pass


def build():
    nc = bacc.Bacc("TRN2", target_bir_lowering=False, debug=False)

    xT = nc.declare_dram_parameter("xT", [BPC, C, S], ADT, isOutput=False)
    wqkvT = nc.declare_dram_parameter("wqkvT", [C, 3 * C], ADT, isOutput=False)
    wouT = nc.declare_dram_parameter("wouT", [C, C], MDT, isOutput=False)
    bqk = nc.declare_dram_parameter("bqk", [128, FCH], F32, isOutput=False)
    beff = nc.declare_dram_parameter("beff", [C], F32, isOutput=False)
    y = nc.declare_dram_parameter("y", [BPC, S, C], F32, isOutput=True)

    from contextlib import ExitStack

    with tile.TileContext(nc) as tc, ExitStack() as ctx:
        ctx.enter_context(
            nc.allow_low_precision(reason="bf16/f32r matmul operand staging")
        )
        consts = ctx.enter_context(tc.tile_pool(name="consts", bufs=1))
        xpool = ctx.enter_context(tc.tile_pool(name="x", bufs=2))
        qkpool = ctx.enter_context(tc.tile_pool(name="qkt", bufs=12))
        vpool = ctx.enter_context(tc.tile_pool(name="v", bufs=2))
        ppool = ctx.enter_context(tc.tile_pool(name="p", bufs=4))
        opool = ctx.enter_context(tc.tile_pool(name="o", bufs=2))
        rpool = ctx.enter_context(tc.tile_pool(name="r", bufs=2))
        spool = ctx.enter_context(tc.tile_pool(name="s", bufs=2))
        ypool = ctx.enter_context(tc.tile_pool(name="y", bufs=2))
        bcpool = ctx.enter_context(tc.tile_pool(name="bc", bufs=2))
        drpool = ctx.enter_context(tc.tile_pool(name="dr", bufs=2, space="DRAM"))
        ps_a = ctx.enter_context(tc.tile_pool(name="ps_a", bufs=2, space="PSUM"))
        ps_o = ctx.enter_context(tc.tile_pool(name="ps_o", bufs=1, space="PSUM"))
        ps_y = ctx.enter_context(tc.tile_pool(name="ps_y", bufs=2, space="PSUM"))

        # --- constants ---
        wq_sb = consts.tile([128, CCH * 3 * C], ADT)  # [c%128, cc*1536 + f]
        for cc in range(CCH):
            nc.sync.dma_start(
                out=wq_sb[:, cc * 1536 : (cc + 1) * 1536],
                in_=wqkvT[cc * 128 : (cc + 1) * 128, :],
            )
        wo_sb = consts.tile([128, CCH * C], MDT)  # [c%128, cc*512 + f]
        nc.sync.dma_start(
            out=wo_sb.rearrange("p (cc f) -> p cc f", cc=CCH),
            in_=wouT[:, :].rearrange("(cc p) f -> p cc f", p=128),
        )
        bqk_sb = consts.tile([128, FCH], F32)
        nc.sync.dma_start(out=bqk_sb, in_=bqk[:, :])
        beff_sb = consts.tile([128, C], F32)  # b_eff broadcast to all partitions
        nc.gpsimd.dma_start(out=beff_sb, in_=beff[:].partition_broadcast(128))

        def emit_x(b):
            # x^T for batch b: [c, s] as [c%128, cc*1024 + s]
            x_sb = xpool.tile([128, CCH * S], ADT, tag="x", name=f"x{b}")
            for cc in range(CCH):
                nc.sync.dma_start(
                    out=x_sb[:, cc * S : (cc + 1) * S],
                    in_=xT[b][cc * 128 : (cc + 1) * 128, :],
                )
            return x_sb

        def emit_qk_chunk(b, x_sb, fc):
            # q/k projection chunk: qkT[fc] = W_qk^T[:,fc].T @ x^T + b
            qt = qkpool.tile([128, S], ADT, tag="qkt", name=f"qkt{b}_{fc}")
            ps = ps_a.tile([128, 1024], F32, tag="ps_a", name=f"psq{b}_{fc}")
            for ih in range(2):
                for cc in range(CCH):
                    nc.tensor.matmul(
                        ps[:, ih * 512 : (ih + 1) * 512],
                        lhsT=wq_sb[:, cc * 1536 + fc * 128 : cc * 1536 + (fc + 1) * 128],
                        rhs=x_sb[:, cc * S + ih * 512 : cc * S + ih * 512 + 512],
                        start=(cc == 0),
                        stop=(cc == CCH - 1),
                    )
            # evacuate + bias (per-partition scalar add), cast to bf16
            nc.vector.tensor_scalar_add(
                out=qt, in0=ps[:, :], scalar1=bqk_sb[:, fc : fc + 1]
            )
            return qt

        def emit_v(b, x_sb):
            # v projection: token-major [s%128, jc*520 + h*65 + d], ones cols
            v_sb = vpool.tile([128, SCJ * VW], ADT, tag="v", name=f"v{b}")
            v_view = v_sb.rearrange("p (jc h dd) -> p jc h dd", jc=SCJ, h=H)
            nc.gpsimd.memset(v_sb, 1.0)  # ones cols survive the data copies
            for jc in range(SCJ):
                ps = ps_a.tile([128, 1024], F32, tag="ps_a", name=f"psv{b}_{jc}")
                for cc in range(CCH):
                    nc.tensor.matmul(
                        ps[:, 0:512],
                        lhsT=x_sb[:, cc * S + jc * 128 : cc * S + (jc + 1) * 128],
                        rhs=wq_sb[:, cc * 1536 + 1024 : cc * 1536 + 1536],
                        start=(cc == 0),
                        stop=(cc == CCH - 1),
                    )
                nc.vector.tensor_copy(
                    out=v_view[:, jc, :, 0:D],
                    in_=ps[:, 0:512].rearrange("p (h d) -> p h d", h=H),
                )
            return v_sb

        def emit_head(b, h, qk_tiles, v_sb, o_sb, sums_sb):
            fq = h // 2  # q features chunk
            fk = 4 + h // 2  # k features chunk
            pb = (h % 2) * 64  # partition base within chunk
            po = ps_o.tile([65, 1024], F32, tag="ps_o", name=f"po{b}_{h}")
            for jc in range(SCJ):
                # scores S'[j, i] = k . q  (transposed scores)
                ps = ps_a.tile([128, 1024], F32, tag="ps_a", name=f"pss{b}_{h}_{jc}")
                for ih in range(2):
                    nc.tensor.matmul(
                        ps[:, ih * 512 : (ih + 1) * 512],
                        lhsT=qk_tiles[fk][pb : pb + 64, jc * 128 : (jc + 1) * 128],
                        rhs=qk_tiles[fq][pb : pb + 64, ih * 512 : ih * 512 + 512],
                        start=True,
                        stop=True,
                    )
                # P' = exp(scale * S')
                pt = ppool.tile([128, 1024], ADT, tag="p", name=f"pt{b}_{h}_{jc}")
                nc.scalar.activation(
                    out=pt, in_=ps[:, :],
                    func=mybir.ActivationFunctionType.Exp,
                    scale=float(D) ** -0.5,
                )
                # O^T[d, i] += V_ext^T @ P'  (row 64 = row-sums)
                for ih in range(2):
                    nc.tensor.matmul(
                        po[:, ih * 512 : (ih + 1) * 512],
                        lhsT=v_sb[:, jc * VW + h * (D + 1) : jc * VW + (h + 1) * (D + 1)],
                        rhs=pt[:, ih * 512 : (ih + 1) * 512],
                        start=(jc == 0),
                        stop=(jc == SCJ - 1),
                    )
            # evacuate unnormalized O^T and row-sums (frees PSUM quickly)
            hh = h % 2
            nc.vector.tensor_copy(
                out=o_sb[hh * 64 : (hh + 1) * 64, (h // 2) * S : (h // 2 + 1) * S],
                in_=po[0:64, :],
            )
            nc.vector.tensor_copy(
                out=sums_sb[32 * hh : 32 * hh + 1, :], in_=po[64:65, :]
            )

        def emit_normalize(b, hp, o_sb, sums_sb):
            # normalize pair hp: bounce sums through DRAM to respread onto
            # 128 lanes (reciprocal is ~8 cyc/elem/lane), broadcast back,
            # multiply in place. Hides under later pairs' attention.
            sums_dr = drpool.tile([2 * S], F32, tag="sdr", name=f"sdr{b}_{hp}")
            for i in range(2):
                nc.sync.dma_start(
                    out=sums_dr[i * S : (i + 1) * S].unsqueeze(0),
                    in_=sums_sb[32 * i : 32 * i + 1, :],
                )
            sums_sq = rpool.tile([128, 2 * S // 128], F32, tag="ssq", name=f"ssq{b}_{hp}")
            nc.sync.dma_start(
                out=sums_sq, in_=sums_dr.rearrange("(p c) -> p c", p=128)
            )
            recs_sq = rpool.tile([128, 2 * S // 128], F32, tag="rsq", name=f"rsq{b}_{hp}")
            nc.vector.reciprocal(out=recs_sq, in_=sums_sq)
            recs_dr = drpool.tile([2 * S], F32, tag="rdr", name=f"rdr{b}_{hp}")
            nc.sync.dma_start(
                out=recs_dr.rearrange("(p c) -> p c", p=128), in_=recs_sq
            )
            bc = bcpool.tile([128, S], F32, tag="bc", name=f"bc{b}_{hp}")
            for i in range(2):
                nc.sync.dma_start(
                    out=bc[i * 64 : (i + 1) * 64, :],
                    in_=recs_dr[i * S : (i + 1) * S].partition_broadcast(64),
                )
            nc.vector.tensor_mul(
                out=o_sb[:, hp * S : (hp + 1) * S],
                in0=o_sb[:, hp * S : (hp + 1) * S],
                in1=bc,
            )

        def emit_outproj_partial(b, hp, o_sb, y_sb):
            # accumulate this pair's contribution to y right after its
            # normalize, spreading the output projection across attention
            for sc in range(SCJ):
                ps = ps_y.tile([128, 512], F32, tag="ps_y", name=f"psy{b}_{hp}_{sc}")
                nc.tensor.matmul(
                    ps[:, 0:512],
                    lhsT=o_sb[:, hp * S + sc * 128 : hp * S + (sc + 1) * 128],
                    rhs=wo_sb[:, hp * C : (hp + 1) * C],
                    start=True,
                    stop=True,
                )
                nc.vector.tensor_add(
                    out=y_sb[:, sc * C : (sc + 1) * C],
                    in0=ps[:, 0:512],
                    in1=beff_sb if hp == 0 else y_sb[:, sc * C : (sc + 1) * C],
                )
                if hp == CCH - 1:
                    nc.sync.dma_start(
                        out=y[b][sc * 128 : (sc + 1) * 128, :],
                        in_=y_sb[:, sc * C : (sc + 1) * C],
                    )

        # --- batch-pipelined schedule: the next batch's full projection is
        # emitted between this batch's attention and output projection, so it
        # fills the PE while the final pair's normalize chain completes.
        def emit_proj(b):
            x_sb = emit_x(b)
            qk = [emit_qk_chunk(b, x_sb, fc) for fc in range(FCH)]
            v_sb = emit_v(b, x_sb)
            return x_sb, qk, v_sb

        state = emit_proj(0)
        for b in range(BPC):
            x_sb, qk_tiles, v_sb = state
            o_sb = opool.tile([128, CCH * S], MDT, tag="o", name=f"o{b}")
            y_sb = ypool.tile([128, SCJ * C], F32, tag="y", name=f"y{b}")
            for hp in range(H // 2):
                sums_sb = spool.tile([33, 1024], F32, tag="sums", name=f"sm{b}_{hp}")
                emit_head(b, 2 * hp, qk_tiles, v_sb, o_sb, sums_sb)
                emit_head(b, 2 * hp + 1, qk_tiles, v_sb, o_sb, sums_sb)
                emit_normalize(b, hp, o_sb, sums_sb)
                emit_outproj_partial(b, hp, o_sb, y_sb)
            if b + 1 < BPC:
                state = emit_proj(b + 1)

    nc.compile()
    return nc


_NC_CACHE = None
LAST_RESULT = None


def kernel(vis_feat, text_feat, w_qkv, b_qkv, w_out, b_out):
    global _NC_CACHE, LAST_RESULT
    _register_ntff_hook()
    if _NC_CACHE is None:
        _NC_CACHE = build()
    nc = _NC_CACHE

    adt_np = np.dtype(mybir.dt.np(ADT))
    vis_feat = np.asarray(vis_feat, dtype=np.float32)
    w_qkv = np.asarray(w_qkv, dtype=np.float32)
    b_qkv = np.asarray(b_qkv, dtype=np.float32)
    w_out = np.asarray(w_out, dtype=np.float32)
    b_out = np.asarray(b_out, dtype=np.float32)

    wqkvT = np.ascontiguousarray(w_qkv.T).astype(adt_np)  # [C, 3C]
    wouT = np.ascontiguousarray(w_out.T)  # [C, C] (f32 bits for f32r)
    bqk = np.ascontiguousarray(b_qkv[: 2 * C].reshape(FCH, 128).T)  # [128, 8]
    beff = np.ascontiguousarray(b_out + b_qkv[2 * C :] @ w_out.T)  # [C]

    in_maps = []
    for i in range(NCORES):
        xTi = np.ascontiguousarray(
            vis_feat[i * BPC : (i + 1) * BPC].transpose(0, 2, 1)
        ).astype(adt_np)  # [BPC, C, S]
        in_maps.append(
            {"xT": xTi, "wqkvT": wqkvT, "wouT": wouT, "bqk": bqk, "beff": beff}
        )

    res = run_bass_kernel_spmd(nc, in_maps, core_ids=list(range(NCORES)))
    LAST_RESULT = res
    return np.concatenate([res.results[i]["y"] for i in range(NCORES)], axis=0)


# revision 34
# speedup vs baseline: 1.3881x; 1.3881x over previous
"""Self-attention block (B=16, S=1024, C=512, H=8, D=64) on 8 NeuronCores.

Data-parallel over batch: core i handles batches [2i, 2i+1]. No collectives.

Per-core device pipeline (all on-chip after the initial DMAs):
  qkv proj -> q,k feature-major [d, s], v token-major [s, d] with a ones
  column appended per head (so P@V_ext also yields the softmax row-sums);
  scores computed transposed S'[j, i] = k . q so exp(S') feeds the P@V
  matmul directly as lhsT with no transposes; softmax skips max-subtraction
  (logits bounded ~+-4, mathematically identical); deferred normalization
  divides O^T rows by the row-sum (reciprocal respread over 128 lanes via a
  DRAM bounce, broadcast back, in-place multiply); output projection
  consumes the normalized heads straight out of SBUF.

The value-path bias is folded through attention into the output bias
(b_eff = b_out + b_v @ w_out.T), exact because softmax rows sum to 1.

Scheduling: the next batch's q/k/v projection chunks are emitted between
this batch's attention head-pairs so the PE's slack under the ACT-bound
exp stream absorbs them.

Dtypes: attention core + q/k/v projections run bf16 (PE streams 2
elem/cycle); the output projection runs float32r (single-pass fp32). Set
ATTN_QK_DT=f32r for an all-f32r kernel (~1.4e-4 max rel err vs ~1.1e-3).
"""

import os
import numpy as np

import concourse.bacc as bacc
import concourse.tile as tile
import concourse.mybir as mybir
from concourse.bass_utils import run_bass_kernel_spmd

B, S, C, H, D = 16, 1024, 512, 8, 64
NCORES = 8
BPC = B // NCORES  # batches per core
F32 = mybir.dt.float32
MDT = mybir.dt.float32r if os.environ.get("ATTN_MM_DT", "f32r") == "f32r" else F32
ADT = mybir.dt.bfloat16 if os.environ.get("ATTN_QK_DT", "bf16") == "bf16" else MDT

SCJ = 8  # S/128 chunks (token/key chunks)
CCH = 4  # C/128 chunks (model-dim chunks)
FCH = 8  # (2C)/128 chunks of q|k features
VW = H * (D + 1)  # 520: v row width incl. ones column per head


def _register_ntff_hook():
    # run_bass_kernel_spmd(trace=True) under axon needs antenv.axon_hooks,
    # which is absent in this image; register the equivalent hook directly.
    import sys, types

    if "antenv.axon_hooks" in sys.modules:
        return
    try:
        import trn_agent_boot.trn_boot as tb

        hook = [None]
        mod = types.ModuleType("antenv.axon_hooks")
        mod.set_axon_ntff_profile_hook = lambda h: hook.__setitem__(0, h)
        mod.get_axon_ntff_profile_hook = lambda: hook[0]
        sys.modules["antenv.axon_hooks"] = mod
        mod.set_axon_ntff_profile_hook(
            tb._ntff_profile_via_ctypes("/opt/axon/libaxon_pjrt.so")
        )
    except Exception:
        pass


def build():
    nc = bacc.Bacc("TRN2", target_bir_lowering=False, debug=False)

    xT = nc.declare_dram_parameter("xT", [BPC, C, S], ADT, isOutput=False)
    wqkvT = nc.declare_dram_parameter("wqkvT", [C, 3 * C], ADT, isOutput=False)
    wouT = nc.declare_dram_parameter("wouT", [C, C], MDT, isOutput=False)
    bqk = nc.declare_dram_parameter("bqk", [128, FCH], F32, isOutput=False)
    beff = nc.declare_dram_parameter("beff", [C], F32, isOutput=False)
    y = nc.declare_dram_parameter("y", [BPC, S, C], F32, isOutput=True)

    from contextlib import ExitStack

    with tile.TileContext(nc) as tc, ExitStack() as ctx:
        ctx.enter_context(
            nc.allow_low_precision(reason="bf16/f32r matmul operand staging")
        )
        consts = ctx.enter_context(tc.tile_pool(name="consts", bufs=1))
        xpool = ctx.enter_context(tc.tile_pool(name="x", bufs=2))
        qkpool = ctx.enter_context(tc.tile_pool(name="qkt", bufs=12))
        vpool = ctx.enter_context(tc.tile_pool(name="v", bufs=2))
        ppool = ctx.enter_context(tc.tile_pool(name="p", bufs=4))
        opool = ctx.enter_context(tc.tile_pool(name="o", bufs=2))
        rpool = ctx.enter_context(tc.tile_pool(name="r", bufs=2))
        spool = ctx.enter_context(tc.tile_pool(name="s", bufs=2))
        ypool = ctx.enter_context(tc.tile_pool(name="y", bufs=2))
        bcpool = ctx.enter_context(tc.tile_pool(name="bc", bufs=2))
        drpool = ctx.enter_context(tc.tile_pool(name="dr", bufs=2, space="DRAM"))
        ps_a = ctx.enter_context(tc.tile_pool(name="ps_a", bufs=3, space="PSUM"))
        ps_o = ctx.enter_context(tc.tile_pool(name="ps_o", bufs=1, space="PSUM"))

        # --- constants ---
        wq_sb = consts.tile([128, CCH * 3 * C], ADT)  # [c%128, cc*1536 + f]
        for cc in range(CCH):
            nc.sync.dma_start(
                out=wq_sb[:, cc * 1536 : (cc + 1) * 1536],
                in_=wqkvT[cc * 128 : (cc + 1) * 128, :],
            )
        wo_sb = consts.tile([128, CCH * C], MDT)  # [c%128, cc*512 + f]
        nc.sync.dma_start(
            out=wo_sb.rearrange("p (cc f) -> p cc f", cc=CCH),
            in_=wouT[:, :].rearrange("(cc p) f -> p cc f", p=128),
        )
        bqk_sb = consts.tile([128, FCH], F32)
        nc.sync.dma_start(out=bqk_sb, in_=bqk[:, :])
        beff_sb = consts.tile([128, C], F32)  # b_eff broadcast to all partitions
        nc.gpsimd.dma_start(out=beff_sb, in_=beff[:].partition_broadcast(128))

        def emit_x(b):
            # x^T for batch b: [c, s] as [c%128, cc*1024 + s]
            x_sb = xpool.tile([128, CCH * S], ADT, tag="x", name=f"x{b}")
            for cc in range(CCH):
                nc.sync.dma_start(
                    out=x_sb[:, cc * S : (cc + 1) * S],
                    in_=xT[b][cc * 128 : (cc + 1) * 128, :],
                )
            return x_sb

        def emit_qk_chunk(b, x_sb, fc):
            # q/k projection chunk: qkT[fc] = W_qk^T[:,fc].T @ x^T + b
            qt = qkpool.tile([128, S], ADT, tag="qkt", name=f"qkt{b}_{fc}")
            ps = ps_a.tile([128, 1024], F32, tag="ps_a", name=f"psq{b}_{fc}")
            for ih in range(2):
                for cc in range(CCH):
                    nc.tensor.matmul(
                        ps[:, ih * 512 : (ih + 1) * 512],
                        lhsT=wq_sb[:, cc * 1536 + fc * 128 : cc * 1536 + (fc + 1) * 128],
                        rhs=x_sb[:, cc * S + ih * 512 : cc * S + ih * 512 + 512],
                        start=(cc == 0),
                        stop=(cc == CCH - 1),
                    )
            # evacuate + bias (per-partition scalar add), cast to bf16
            nc.vector.tensor_scalar_add(
                out=qt, in0=ps[:, :], scalar1=bqk_sb[:, fc : fc + 1]
            )
            return qt

        def emit_v(b, x_sb):
            # v projection: token-major [s%128, jc*520 + h*65 + d], ones cols
            v_sb = vpool.tile([128, SCJ * VW], ADT, tag="v", name=f"v{b}")
            v_view = v_sb.rearrange("p (jc h dd) -> p jc h dd", jc=SCJ, h=H)
            nc.gpsimd.memset(v_sb, 1.0)  # ones cols survive the data copies
            for jc in range(SCJ):
                ps = ps_a.tile([128, 1024], F32, tag="ps_a", name=f"psv{b}_{jc}")
                for cc in range(CCH):
                    nc.tensor.matmul(
                        ps[:, 0:512],
                        lhsT=x_sb[:, cc * S + jc * 128 : cc * S + (jc + 1) * 128],
                        rhs=wq_sb[:, cc * 1536 + 1024 : cc * 1536 + 1536],
                        start=(cc == 0),
                        stop=(cc == CCH - 1),
                    )
                nc.vector.tensor_copy(
                    out=v_view[:, jc, :, 0:D],
                    in_=ps[:, 0:512].rearrange("p (h d) -> p h d", h=H),
                )
            return v_sb

        def emit_head(b, h, qk_tiles, v_sb, o_sb, sums_sb):
            fq = h // 2  # q features chunk
            fk = 4 + h // 2  # k features chunk
            pb = (h % 2) * 64  # partition base within chunk
            po = ps_o.tile([65, 1024], F32, tag="ps_o", name=f"po{b}_{h}")
            for jc in range(SCJ):
                # scores S'[j, i] = k . q  (transposed scores)
                ps = ps_a.tile([128, 1024], F32, tag="ps_a", name=f"pss{b}_{h}_{jc}")
                for ih in range(2):
                    nc.tensor.matmul(
                        ps[:, ih * 512 : (ih + 1) * 512],
                        lhsT=qk_tiles[fk][pb : pb + 64, jc * 128 : (jc + 1) * 128],
                        rhs=qk_tiles[fq][pb : pb + 64, ih * 512 : ih * 512 + 512],
                        start=True,
                        stop=True,
                    )
                # P' = exp(scale * S')
                pt = ppool.tile([128, 1024], ADT, tag="p", name=f"pt{b}_{h}_{jc}")
                nc.scalar.activation(
                    out=pt, in_=ps[:, :],
                    func=mybir.ActivationFunctionType.Exp,
                    scale=float(D) ** -0.5,
                )
                # O^T[d, i] += V_ext^T @ P'  (row 64 = row-sums)
                for ih in range(2):
                    nc.tensor.matmul(
                        po[:, ih * 512 : (ih + 1) * 512],
                        lhsT=v_sb[:, jc * VW + h * (D + 1) : jc * VW + (h + 1) * (D + 1)],
                        rhs=pt[:, ih * 512 : (ih + 1) * 512],
                        start=(jc == 0),
                        stop=(jc == SCJ - 1),
                    )
            # evacuate unnormalized O^T and row-sums (frees PSUM quickly)
            hh = h % 2
            nc.vector.tensor_copy(
                out=o_sb[hh * 64 : (hh + 1) * 64, (h // 2) * S : (h // 2 + 1) * S],
                in_=po[0:64, :],
            )
            nc.vector.tensor_copy(
                out=sums_sb[32 * hh : 32 * hh + 1, :], in_=po[64:65, :]
            )

        def emit_normalize(b, hp, o_sb, sums_sb):
            # normalize pair hp: bounce sums through DRAM to respread onto
            # 128 lanes (reciprocal is ~8 cyc/elem/lane), broadcast back,
            # multiply in place. Hides under later pairs' attention.
            sums_dr = drpool.tile([2 * S], F32, tag="sdr", name=f"sdr{b}_{hp}")
            for i in range(2):
                nc.sync.dma_start(
                    out=sums_dr[i * S : (i + 1) * S].unsqueeze(0),
                    in_=sums_sb[32 * i : 32 * i + 1, :],
                )
            sums_sq = rpool.tile([128, 2 * S // 128], F32, tag="ssq", name=f"ssq{b}_{hp}")
            nc.sync.dma_start(
                out=sums_sq, in_=sums_dr.rearrange("(p c) -> p c", p=128)
            )
            recs_sq = rpool.tile([128, 2 * S // 128], F32, tag="rsq", name=f"rsq{b}_{hp}")
            nc.vector.reciprocal(out=recs_sq, in_=sums_sq)
            recs_dr = drpool.tile([2 * S], F32, tag="rdr", name=f"rdr{b}_{hp}")
            nc.sync.dma_start(
                out=recs_dr.rearrange("(p c) -> p c", p=128), in_=recs_sq
            )
            bc = bcpool.tile([128, S], F32, tag="bc", name=f"bc{b}_{hp}")
            for i in range(2):
                nc.sync.dma_start(
                    out=bc[i * 64 : (i + 1) * 64, :],
                    in_=recs_dr[i * S : (i + 1) * S].partition_broadcast(64),
                )
            nc.vector.tensor_mul(
                out=o_sb[:, hp * S : (hp + 1) * S],
                in0=o_sb[:, hp * S : (hp + 1) * S],
                in1=bc,
            )

        def emit_outproj(b, o_sb):
            y_sb = ypool.tile([128, SCJ * C], F32, tag="y", name=f"y{b}")
            for sc in range(SCJ):
                ps = ps_a.tile([128, 1024], F32, tag="ps_a", name=f"psy{b}_{sc}")
                for cc in range(CCH):
                    nc.tensor.matmul(
                        ps[:, 0:512],
                        lhsT=o_sb[:, cc * S + sc * 128 : cc * S + (sc + 1) * 128],
                        rhs=wo_sb[:, cc * C : (cc + 1) * C],
                        start=(cc == 0),
                        stop=(cc == CCH - 1),
                    )
                nc.vector.tensor_add(
                    out=y_sb[:, sc * C : (sc + 1) * C],
                    in0=ps[:, 0:512],
                    in1=beff_sb,
                )
                nc.sync.dma_start(
                    out=y[b][sc * 128 : (sc + 1) * 128, :],
                    in_=y_sb[:, sc * C : (sc + 1) * C],
                )

        # --- batch-pipelined schedule: the next batch's full projection is
        # emitted between this batch's attention and output projection, so it
        # fills the PE while the final pair's normalize chain completes.
        def emit_proj(b):
            x_sb = emit_x(b)
            qk = [emit_qk_chunk(b, x_sb, fc) for fc in range(FCH)]
            v_sb = emit_v(b, x_sb)
            return x_sb, qk, v_sb

        state = emit_proj(0)
        for b in range(BPC):
            x_sb, qk_tiles, v_sb = state
            o_sb = opool.tile([128, CCH * S], MDT, tag="o", name=f"o{b}")
            for hp in range(H // 2):
                sums_sb = spool.tile([33, 1024], F32, tag="sums", name=f"sm{b}_{hp}")
                emit_head(b, 2 * hp, qk_tiles, v_sb, o_sb, sums_sb)
                emit_head(b, 2 * hp + 1, qk_tiles, v_sb, o_sb, sums_sb)
                emit_normalize(b, hp, o_sb, sums_sb)
            if b + 1 < BPC:
                state = emit_proj(b + 1)
            emit_outproj(b, o_sb)

    nc.compile()
    return nc


_NC_CACHE = None
LAST_RESULT = None


def kernel(vis_feat, text_feat, w_qkv, b_qkv, w_out, b_out):
    global _NC_CACHE, LAST_RESULT
    _register_ntff_hook()
    if _NC_CACHE is None:
        _NC_CACHE = build()
    nc = _NC_CACHE

    adt_np = np.dtype(mybir.dt.np(ADT))
    vis_feat = np.asarray(vis_feat, dtype=np.float32)
    w_qkv = np.asarray(w_qkv, dtype=np.float32)
    b_qkv = np.asarray(b_qkv, dtype=np.float32)
    w_out = np.asarray(w_out, dtype=np.float32)
    b_out = np.asarray(b_out, dtype=np.float32)

    wqkvT = np.ascontiguousarray(w_qkv.T).astype(adt_np)  # [C, 3C]
    wouT = np.ascontiguousarray(w_out.T)  # [C, C] (f32 bits for f32r)
    bqk = np.ascontiguousarray(b_qkv[: 2 * C].reshape(FCH, 128).T)  # [128, 8]
    beff = np.ascontiguousarray(b_out + b_qkv[2 * C :] @ w_out.T)  # [C]

    in_maps = []
    for i in range(NCORES):
        xTi = np.ascontiguousarray(
            vis_feat[i * BPC : (i + 1) * BPC].transpose(0, 2, 1)
        ).astype(adt_np)  # [BPC, C, S]
        in_maps.append(
            {"xT": xTi, "wqkvT": wqkvT, "wouT": wouT, "bqk": bqk, "beff": beff}
        )

    res = run_bass_kernel_spmd(nc, in_maps, core_ids=list(range(NCORES)))
    LAST_RESULT = res
    return np.concatenate([res.results[i]["y"] for i in range(NCORES)], axis=0)
